# revision 5
# baseline (speedup 1.0000x reference)
import sys, os
sys.path.insert(0, "/opt/trn_rl_repo")
import numpy as np
from contextlib import ExitStack

import concourse.bass as bass
import concourse.mybir as mybir
from concourse.mybir import AluOpType as OP
from concourse.bass_utils import run_bass_kernel_spmd

f32 = mybir.dt.float32
EPS = 1e-5
B, TX, S, T, R = 16, 4, 3276, 14, 4
ST = S * T                      # 45864 resource elements per batch
NB = 2                          # batches per core
P = 126                         # partitions
FB = ST // P                    # 364 lanes per partition per batch
NCH = 2                         # free-dim chunks per batch
F = FB // NCH                   # 182
NCORES = 8
NPL = 176                       # work planes
TRACE = False
LAST_EXEC_NS = None

# ------------------------------------------------------------------ kernel build
def build_nc():
    nc = bass.Bass("TRN2")
    h_d = nc.dram_tensor("h", [NB, TX, ST, 8], f32, kind="ExternalInput")
    yr_d = nc.dram_tensor("yr", [NB, ST, 4], f32, kind="ExternalInput")
    yi_d = nc.dram_tensor("yi", [NB, ST, 4], f32, kind="ExternalInput")
    s_d = nc.dram_tensor("s", [NB, ST, 16], f32, kind="ExternalInput")
    a_d = nc.dram_tensor("act", [NB, TX, ST], f32, kind="ExternalInput")
    sc_d = nc.dram_tensor("scal", [P, 4], f32, kind="ExternalInput")   # gamma, theta, -theta, zeta
    x_d = nc.dram_tensor("x", [NB, TX, ST, 2], f32, kind="ExternalOutput")
    no_d = nc.dram_tensor("no", [NB, TX, ST], f32, kind="ExternalOutput")

    with ExitStack() as ctx:
        hin = ctx.enter_context(nc.sbuf_tensor([P, TX * F * 8], f32))
        yrin = ctx.enter_context(nc.sbuf_tensor([P, F * 4], f32))
        yiin = ctx.enter_context(nc.sbuf_tensor([P, F * 4], f32))
        sin = ctx.enter_context(nc.sbuf_tensor([P, F * 16], f32))
        ain = ctx.enter_context(nc.sbuf_tensor([P, TX * F], f32))
        scal = ctx.enter_context(nc.sbuf_tensor([P, 4], f32))
        xout = ctx.enter_context(nc.sbuf_tensor([P, TX * F * 2], f32))
        nout = ctx.enter_context(nc.sbuf_tensor([P, TX * F], f32))
        work = ctx.enter_context(nc.sbuf_tensor([P, NPL * F], f32))
        dsem_in = ctx.enter_context(nc.semaphore())
        dsem_out = ctx.enter_context(nc.semaphore())
        vsem = ctx.enter_context(nc.semaphore())
        block = ctx.enter_context(nc.Block())

        CHUNKS = [(b, c) for b in range(NB) for c in range(NCH)]

        @block.sync
        def _(sync):
            for k, (b, c) in enumerate(CHUNKS):
                if k > 0:
                    sync.wait_ge(vsem, k)      # vector done reading chunk k-1 inputs
                # loads: partition p covers st = p*FB + c*F + l
                hv = h_d[b].rearrange("i (p c l) v -> p c i (l v)", p=P, c=NCH, l=F)[:, c]
                sync.dma_start(hin[:].rearrange("p (i m) -> p i m", i=TX), hv).then_inc(dsem_in, 16)
                sync.dma_start(yrin[:], yr_d[b].rearrange("(p c l) v -> p c (l v)", p=P, c=NCH, l=F)[:, c]).then_inc(dsem_in, 16)
                sync.dma_start(yiin[:], yi_d[b].rearrange("(p c l) v -> p c (l v)", p=P, c=NCH, l=F)[:, c]).then_inc(dsem_in, 16)
                sync.dma_start(sin[:], s_d[b].rearrange("(p c l) v -> p c (l v)", p=P, c=NCH, l=F)[:, c]).then_inc(dsem_in, 16)
                sync.dma_start(ain[:].rearrange("p (i l) -> p i l", i=TX), a_d[b].rearrange("i (p c l) -> p c i l", p=P, c=NCH, l=F)[:, c]).then_inc(dsem_in, 16)
                if k == 0:
                    sync.dma_start(scal[:], sc_d[:, :]).then_inc(dsem_in, 16)
                sync.wait_ge(vsem, k + 1)      # vector finished chunk k outputs
                xv = x_d[b].rearrange("i (p c l) v -> p c i (l v)", p=P, c=NCH, l=F)[:, c]
                sync.dma_start(xv, xout[:].rearrange("p (i m) -> p i m", i=TX)).then_inc(dsem_out, 16)
                sync.dma_start(no_d[b].rearrange("i (p c l) -> p c i l", p=P, c=NCH, l=F)[:, c],
                               nout[:].rearrange("p (i l) -> p i l", i=TX)).then_inc(dsem_out, 16)

        def emit_chunk(nc):
            V = nc.vector
            h4 = hin[:].rearrange("p (i l v) -> p i l v", i=TX, l=F)
            s16 = sin[:].rearrange("p (l v) -> p l v", l=F)
            yr4 = yrin[:].rearrange("p (l v) -> p l v", l=F)
            yi4 = yiin[:].rearrange("p (l v) -> p l v", l=F)
            a3 = ain[:].rearrange("p (i l) -> p i l", i=TX)
            x4 = xout[:].rearrange("p (i l v) -> p i l v", i=TX, l=F)
            n3 = nout[:].rearrange("p (i l) -> p i l", i=TX)
            hr = lambda i, a: h4[:, i, :, a]
            hi = lambda i, a: h4[:, i, :, 4 + a]
            sab = lambda a, bb: s16[:, :, 4 * a + bb]
            gamma, theta, ntheta, zeta = (scal[:, j:j + 1] for j in range(4))

            cnt = [0]
            def pl():
                i = cnt[0]; cnt[0] += 1
                assert i < NPL
                return work[:, i * F:(i + 1) * F]

            def MUL(o, x, y): V.tensor_tensor(o, x, y, OP.mult)
            def ADD(o, x, y): V.tensor_tensor(o, x, y, OP.add)
            def SUB(o, x, y): V.tensor_tensor(o, x, y, OP.subtract)

            t1, t2, t3, t4 = pl(), pl(), pl(), pl()

            # --- n_i[a] = hr^2 + hi^2 ; P/Q products for pairs
            n = {}
            for i in range(TX):
                for a in range(R):
                    n[(i, a)] = pl()
                    MUL(t1, hr(i, a), hr(i, a)); MUL(t2, hi(i, a), hi(i, a))
                    ADD(n[(i, a)], t1, t2)
            PAIRS = [(0, 1), (0, 2), (0, 3), (1, 2), (1, 3), (2, 3)]
            Pp, Qp = {}, {}
            for (a, bb) in PAIRS:
                for i in range(TX):
                    Pp[(i, a, bb)] = pl(); Qp[(i, a, bb)] = pl()
                    MUL(t1, hr(i, a), hr(i, bb)); MUL(t2, hi(i, a), hi(i, bb))
                    ADD(Pp[(i, a, bb)], t1, t2)
                    MUL(t1, hi(i, a), hr(i, bb)); MUL(t2, hr(i, a), hi(i, bb))
                    SUB(Qp[(i, a, bb)], t1, t2)

            # --- G entries: gd[a] real diag; (Gr, Gi) for pairs
            gd = {}
            for a in range(R):
                gd[a] = pl()
                ADD(t1, n[(0, a)], n[(1, a)]); ADD(t2, n[(2, a)], n[(3, a)])
                ADD(t3, t1, t2)
                V.tensor_scalar(t4, sab(a, a), gamma, 0.0, OP.mult, OP.max)
                V.tensor_scalar(t4, t4, EPS, None, OP.add)
                ADD(gd[a], t3, t4)
            Gr, Gi = {}, {}
            for (a, bb) in PAIRS:
                Gr[(a, bb)] = pl(); Gi[(a, bb)] = pl()
                ADD(t1, Pp[(0, a, bb)], Pp[(1, a, bb)]); ADD(t2, Pp[(2, a, bb)], Pp[(3, a, bb)])
                ADD(t3, t1, t2)
                V.tensor_scalar(t4, sab(a, bb), gamma, 0.0, OP.mult, OP.max)
                V.tensor_scalar(t4, t4, EPS, None, OP.add)
                ADD(Gr[(a, bb)], t3, t4)
                ADD(t1, Qp[(0, a, bb)], Qp[(1, a, bb)]); ADD(t2, Qp[(2, a, bb)], Qp[(3, a, bb)])
                ADD(Gi[(a, bb)], t1, t2)

            # --- Schur 2x2-block inverse of G. Blocks: A=rows{0,1}, C=rows{2,3}
            # invA
            rA, iA11, iA22, p12r, p12i = pl(), pl(), pl(), pl(), pl()
            MUL(t1, Gr[(0, 1)], Gr[(0, 1)]); MUL(t2, Gi[(0, 1)], Gi[(0, 1)])
            ADD(t1, t1, t2)
            MUL(t2, gd[0], gd[1]); SUB(t3, t2, t1)
            V.reciprocal(rA, t3)
            MUL(iA11, gd[1], rA); MUL(iA22, gd[0], rA)
            MUL(p12r, Gr[(0, 1)], rA); MUL(p12i, Gi[(0, 1)], rA)   # iA12 = -(p12r + j p12i)
            # B entries: B[k][j] = G[k, 2+j] (complex): k,j in {0,1}
            Br = lambda k, j: Gr[(k, 2 + j)]
            Bi = lambda k, j: Gi[(k, 2 + j)]
            # T = invA * B  (2x2 complex)
            Tr, Ti = {}, {}
            for j in range(2):
                # T[0][j] = iA11*B0j - p12*B1j
                Tr[(0, j)] = pl(); Ti[(0, j)] = pl()
                MUL(t1, iA11, Br(0, j)); MUL(t2, p12r, Br(1, j)); MUL(t3, p12i, Bi(1, j))
                SUB(t4, t1, t2); ADD(Tr[(0, j)], t4, t3)
                MUL(t1, iA11, Bi(0, j)); MUL(t2, p12r, Bi(1, j)); MUL(t3, p12i, Br(1, j))
                SUB(t4, t1, t2); SUB(Ti[(0, j)], t4, t3)
                # T[1][j] = -conj(p12)*B0j + iA22*B1j
                Tr[(1, j)] = pl(); Ti[(1, j)] = pl()
                MUL(t1, p12r, Br(0, j)); MUL(t2, p12i, Bi(0, j)); MUL(t3, iA22, Br(1, j))
                ADD(t4, t1, t2); SUB(Tr[(1, j)], t3, t4)
                MUL(t1, p12r, Bi(0, j)); MUL(t2, p12i, Br(0, j)); MUL(t3, iA22, Bi(1, j))
                SUB(t4, t1, t2); SUB(Ti[(1, j)], t3, t4)
            # Schur complement Sc = C - B^H T (2x2 hermitian)
            Sc0, Sc1, Scr, Sci = pl(), pl(), pl(), pl()
            MUL(t1, Br(0, 0), Tr[(0, 0)]); MUL(t2, Bi(0, 0), Ti[(0, 0)]); ADD(t3, t1, t2)
            MUL(t1, Br(1, 0), Tr[(1, 0)]); MUL(t2, Bi(1, 0), Ti[(1, 0)]); ADD(t4, t1, t2)
            ADD(t3, t3, t4); SUB(Sc0, gd[2], t3)
            MUL(t1, Br(0, 1), Tr[(0, 1)]); MUL(t2, Bi(0, 1), Ti[(0, 1)]); ADD(t3, t1, t2)
            MUL(t1, Br(1, 1), Tr[(1, 1)]); MUL(t2, Bi(1, 1), Ti[(1, 1)]); ADD(t4, t1, t2)
            ADD(t3, t3, t4); SUB(Sc1, gd[3], t3)
            # Sc01 = G23 - sum_k conj(B_k0) T_k1
            MUL(t1, Br(0, 0), Tr[(0, 1)]); MUL(t2, Bi(0, 0), Ti[(0, 1)]); ADD(t3, t1, t2)
            MUL(t1, Br(1, 0), Tr[(1, 1)]); MUL(t2, Bi(1, 0), Ti[(1, 1)]); ADD(t4, t1, t2)
            ADD(t3, t3, t4); SUB(Scr, Gr[(2, 3)], t3)
            MUL(t1, Br(0, 0), Ti[(0, 1)]); MUL(t2, Bi(0, 0), Tr[(0, 1)]); SUB(t3, t1, t2)
            MUL(t1, Br(1, 0), Ti[(1, 1)]); MUL(t2, Bi(1, 0), Tr[(1, 1)]); SUB(t4, t1, t2)
            ADD(t3, t3, t4); SUB(Sci, Gi[(2, 3)], t3)
            # invSc
            rS, iS11, iS22, q12r, q12i = pl(), pl(), pl(), pl(), pl()
            MUL(t1, Scr, Scr); MUL(t2, Sci, Sci)
            ADD(t1, t1, t2)
            MUL(t2, Sc0, Sc1); SUB(t3, t2, t1)
            V.reciprocal(rS, t3)
            MUL(iS11, Sc1, rS); MUL(iS22, Sc0, rS)
            MUL(q12r, Scr, rS); MUL(q12i, Sci, rS)    # iS12 = -(q12r + j q12i)
            # X = -T*invSc : X[k][j], true values. M[0][2]=X00 M[0][3]=X01 M[1][2]=X10 M[1][3]=X11
            Xr, Xi = {}, {}
            for k in range(2):
                Xr[(k, 0)] = pl(); Xi[(k, 0)] = pl()
                # X_k0 = -T_k0*iS11 + T_k1*conj(q12)
                MUL(t1, Tr[(k, 0)], iS11); MUL(t2, Tr[(k, 1)], q12r); MUL(t3, Ti[(k, 1)], q12i)
                ADD(t4, t2, t3); SUB(Xr[(k, 0)], t4, t1)
                MUL(t1, Ti[(k, 0)], iS11); MUL(t2, Ti[(k, 1)], q12r); MUL(t3, Tr[(k, 1)], q12i)
                SUB(t4, t2, t3); SUB(Xi[(k, 0)], t4, t1)
                # X_k1 = T_k0*q12 - T_k1*iS22
                Xr[(k, 1)] = pl(); Xi[(k, 1)] = pl()
                MUL(t1, Tr[(k, 0)], q12r); MUL(t2, Ti[(k, 0)], q12i); MUL(t3, Tr[(k, 1)], iS22)
                SUB(t4, t1, t2); SUB(Xr[(k, 1)], t4, t3)
                MUL(t1, Ti[(k, 0)], q12r); MUL(t2, Tr[(k, 0)], q12i); MUL(t3, Ti[(k, 1)], iS22)
                ADD(t4, t1, t2); SUB(Xi[(k, 1)], t4, t3)
            # M11 block = invA - X*T^H  (hermitian 2x2)
            M00, M11, M01r, M01i = pl(), pl(), pl(), pl()
            MUL(t1, Xr[(0, 0)], Tr[(0, 0)]); MUL(t2, Xi[(0, 0)], Ti[(0, 0)]); ADD(t3, t1, t2)
            MUL(t1, Xr[(0, 1)], Tr[(0, 1)]); MUL(t2, Xi[(0, 1)], Ti[(0, 1)]); ADD(t4, t1, t2)
            ADD(t3, t3, t4); SUB(M00, iA11, t3)
            MUL(t1, Xr[(1, 0)], Tr[(1, 0)]); MUL(t2, Xi[(1, 0)], Ti[(1, 0)]); ADD(t3, t1, t2)
            MUL(t1, Xr[(1, 1)], Tr[(1, 1)]); MUL(t2, Xi[(1, 1)], Ti[(1, 1)]); ADD(t4, t1, t2)
            ADD(t3, t3, t4); SUB(M11, iA22, t3)
            # M01 = iA12 - (X00*conj(T10) + X01*conj(T11)); iA12 = -(p12r+j p12i)
            MUL(t1, Xr[(0, 0)], Tr[(1, 0)]); MUL(t2, Xi[(0, 0)], Ti[(1, 0)]); ADD(t3, t1, t2)
            MUL(t1, Xr[(0, 1)], Tr[(1, 1)]); MUL(t2, Xi[(0, 1)], Ti[(1, 1)]); ADD(t4, t1, t2)
            ADD(t3, t3, t4); ADD(t3, t3, p12r)
            V.tensor_scalar(M01r, t3, -1.0, None, OP.mult)
            MUL(t1, Xi[(0, 0)], Tr[(1, 0)]); MUL(t2, Xr[(0, 0)], Ti[(1, 0)]); SUB(t3, t1, t2)
            MUL(t1, Xi[(0, 1)], Tr[(1, 1)]); MUL(t2, Xr[(0, 1)], Ti[(1, 1)]); SUB(t4, t1, t2)
            ADD(t3, t3, t4); ADD(t3, t3, p12i)
            V.tensor_scalar(M01i, t3, -1.0, None, OP.mult)
            # M23 = -(q12r + j q12i) true planes
            M23r, M23i = pl(), pl()
            V.tensor_scalar(M23r, q12r, -1.0, None, OP.mult)
            V.tensor_scalar(M23i, q12i, -1.0, None, OP.mult)

            # M dict: diag real planes; (a,b) a<b complex true values
            Md = {0: M00, 1: M11, 2: iS11, 3: iS22}
            Mo = {(0, 1): (M01r, M01i), (0, 2): (Xr[(0, 0)], Xi[(0, 0)]),
                  (0, 3): (Xr[(0, 1)], Xi[(0, 1)]), (1, 2): (Xr[(1, 0)], Xi[(1, 0)]),
                  (1, 3): (Xr[(1, 1)], Xi[(1, 1)]), (2, 3): (M23r, M23i)}

            # --- z = M y
            yrp = lambda a: yr4[:, :, a]
            yip = lambda a: yi4[:, :, a]
            z = {}
            for a in range(R):
                zr, zi = pl(), pl()
                MUL(zr, Md[a], yrp(a)); MUL(zi, Md[a], yip(a))
                for bb in range(R):
                    if bb == a:
                        continue
                    if bb > a:
                        mr, mi = Mo[(a, bb)]; sgn = 1.0      # M_ab
                    else:
                        mr, mi = Mo[(bb, a)]; sgn = -1.0     # conj(M_ba)
                    # (mr + j sgn*mi)(yr + j yi): re = mr*yr - sgn*mi*yi ; im = mr*yi + sgn*mi*yr
                    MUL(t1, mr, yrp(bb)); MUL(t2, mi, yip(bb))
                    if sgn > 0:
                        SUB(t3, t1, t2)
                    else:
                        ADD(t3, t1, t2)
                    ADD(zr, zr, t3)
                    MUL(t1, mr, yip(bb)); MUL(t2, mi, yrp(bb))
                    if sgn > 0:
                        ADD(t3, t1, t2)
                    else:
                        SUB(t3, t1, t2)
                    ADD(zi, zi, t3)
                z[a] = (zr, zi)

            # --- gy_i = sum_a conj(H[a,i]) z_a ; d_i ; outputs
            for i in range(TX):
                gyr, gyi = pl(), pl()
                zr, zi = z[0]
                MUL(t1, hr(i, 0), zr); MUL(t2, hi(i, 0), zi); ADD(gyr, t1, t2)
                MUL(t1, hr(i, 0), zi); MUL(t2, hi(i, 0), zr); SUB(gyi, t1, t2)
                for a in range(1, R):
                    zr, zi = z[a]
                    MUL(t1, hr(i, a), zr); MUL(t2, hi(i, a), zi); ADD(t3, t1, t2)
                    ADD(gyr, gyr, t3)
                    MUL(t1, hr(i, a), zi); MUL(t2, hi(i, a), zr); SUB(t3, t1, t2)
                    ADD(gyi, gyi, t3)
                # d_i = sum_a Md[a] n_ia + 2*sum_pairs (P*Mr + Q*Mi)
                dsum, psum = pl(), pl()
                MUL(t1, Md[0], n[(i, 0)]); MUL(t2, Md[1], n[(i, 1)]); ADD(dsum, t1, t2)
                MUL(t1, Md[2], n[(i, 2)]); MUL(t2, Md[3], n[(i, 3)]); ADD(t3, t1, t2)
                ADD(dsum, dsum, t3)
                first = True
                for (a, bb) in PAIRS:
                    mr, mi = Mo[(a, bb)]
                    MUL(t1, Pp[(i, a, bb)], mr); MUL(t2, Qp[(i, a, bb)], mi); ADD(t3, t1, t2)
                    if first:
                        V.tensor_copy(psum, t3); first = False
                    else:
                        ADD(psum, psum, t3)
                # d = dsum + 2*psum ; rd = 1/d
                V.tensor_scalar(t4, psum, 2.0, None, OP.mult)
                ADD(t4, t4, dsum)
                rd = pl()
                V.reciprocal(rd, t4)
                # x_i = gy * rd * act * zeta ; no_eff = relu(theta*rd - theta) + EPS
                V.tensor_scalar(t1, a3[:, i, :], zeta, None, OP.mult)
                MUL(t1, t1, rd)
                MUL(x4[:, i, :, 0], gyr, t1)
                MUL(x4[:, i, :, 1], gyi, t1)
                V.tensor_scalar(t2, rd, theta, ntheta, OP.mult, OP.add)
                last = V.tensor_scalar(n3[:, i, :], t2, 0.0, EPS, OP.max, OP.add)
            return last

        @block.vector
        def _(vector):
            nloads = 0
            for k, (b, c) in enumerate(CHUNKS):
                nloads += 6 if k == 0 else 5
                vector.wait_ge(dsem_in, 16 * nloads)
                if k > 0:
                    vector.wait_ge(dsem_out, 32 * k)   # stores of chunk k-1 done
                emit_chunk(nc).then_inc(vsem, 1)
    return nc


_nc_cache = None


def _get_nc():
    global _nc_cache
    if _nc_cache is None:
        _nc_cache = build_nc()
    return _nc_cache


def kernel(y_real, y_imag, h_hat, s_real, active_tx_x, mcs_ue_mask, gamma, theta, zeta):
    y_real = np.asarray(y_real, dtype=np.float32)
    y_imag = np.asarray(y_imag, dtype=np.float32)
    h_hat = np.asarray(h_hat, dtype=np.float32)
    s_real = np.asarray(s_real, dtype=np.float32)
    active_tx_x = np.asarray(active_tx_x, dtype=np.float32)
    g = float(np.asarray(gamma)); th = float(np.asarray(theta)); ze = float(np.asarray(zeta))

    nc = _get_nc()
    sc = np.tile(np.array([[g, th, -th, ze]], dtype=np.float32), (P, 1))
    in_maps = []
    for cix in range(NCORES):
        bsl = slice(2 * cix, 2 * cix + 2)
        in_maps.append(dict(
            h=np.ascontiguousarray(h_hat[bsl].reshape(NB, TX, ST, 8)),
            yr=np.ascontiguousarray(y_real[bsl].reshape(NB, ST, 4)),
            yi=np.ascontiguousarray(y_imag[bsl].reshape(NB, ST, 4)),
            s=np.ascontiguousarray(s_real[bsl].reshape(NB, ST, 16)),
            act=np.ascontiguousarray(active_tx_x[bsl].reshape(NB, TX, ST)),
            scal=sc,
        ))
    global LAST_EXEC_NS
    res = run_bass_kernel_spmd(nc, in_maps, core_ids=list(range(NCORES)), trace=TRACE)
    LAST_EXEC_NS = res.exec_time_ns
    xs, ns = [], []
    for cix in range(NCORES):
        r = res.results[cix]
        xs.append(r["x"].reshape(NB, TX, S, T, 2))
        ns.append(r["no"].reshape(NB, TX, S, T))
    x = np.concatenate(xs, axis=0)
    x_hat = (x[..., 0] + 1j * x[..., 1]).astype(np.complex64)
    no_eff = np.concatenate(ns, axis=0).astype(np.float32)
    return x_hat, no_eff
